# revision 19
# baseline (speedup 1.0000x reference)
"""Trainium2 Bass kernel for AttentionPatcher (GQA attention block, S=2048).

Sharding: 8-way tensor parallel over KV head groups. Core c owns KV head c
and query heads 4c..4c+3: it computes its Q/K/V projections, RoPE, causal
attention, and a full partial o_proj (wo column shard); a ReduceScatter(add)
over the 8 cores then leaves core c with rows [512c, 512c+512) of the final
output, which the host concatenates.

Schedule: all matmul operands are bf16 (PSUM accumulation stays fp32); wq
and wo stay resident in SBUF. o_proj is computed column-wise (a y s-column
only needs that s-tile's attention outputs), and its matmuls are
interleaved into the NEXT s-tile's attention stream as dependency-free
filler so the PE never stalls on the softmax exp round-trip. The softmax
denominator is accumulated across l-blocks on the vector/gpsimd engines
with a single ones-matmul per (head, s-tile).

PSUM layout: one 6-slot ring ("six") shared by the QKV projection
accumulators / score tiles / o_proj tiles / rope scratch, plus 2 dedicated
slots ("av") for the attention AV accumulators = exactly 8 banks.
"""
import os
import sys

import numpy as np
import ml_dtypes

if os.path.isdir("/opt/trn_rl_repo") and "/opt/trn_rl_repo" not in sys.path:
    sys.path.insert(0, "/opt/trn_rl_repo")

import concourse.bacc as bacc
import concourse.mybir as mybir
import concourse.tile as tile
from concourse.masks import make_identity
from concourse.bass_utils import run_bass_kernel_spmd

F32 = mybir.dt.float32
BF16 = mybir.dt.bfloat16
ActF = mybir.ActivationFunctionType
Alu = mybir.AluOpType
NBF = ml_dtypes.bfloat16

H, KV, D, S = 32, 8, 128, 2048
HID = H * D
NCORES = 8
G = H // KV          # query heads per core
ST = 512             # s-tile size
NST = S // ST        # 4 s-tiles
KO = HID // 128      # 32 contraction subtiles
MO = HID // 128      # 32 output row tiles
INV_SQRT_D = 1.0 / float(np.sqrt(D))
NCHUNK = 4           # ReduceScatter chunks of 8 row-blocks
MO_PER = MO // NCHUNK


def build_nc(with_collective=True):
    nc = bacc.Bacc("TRN2", target_bir_lowering=False, debug=False)

    x = nc.dram_tensor("x", [KO, 128, S], BF16, kind="ExternalInput")
    wq = nc.dram_tensor("wq", [128, KO, G * 128], BF16, kind="ExternalInput")
    wk = nc.dram_tensor("wk", [128, KO, 128], BF16, kind="ExternalInput")
    wv = nc.dram_tensor("wv", [128, KO, 128], BF16, kind="ExternalInput")
    wo = nc.dram_tensor("wo", [128, MO, G * 128], BF16, kind="ExternalInput")
    bq = nc.dram_tensor("bq", [128, G], F32, kind="ExternalInput")
    bk = nc.dram_tensor("bk", [128, 1], F32, kind="ExternalInput")
    bv = nc.dram_tensor("bv", [128, 1], F32, kind="ExternalInput")
    cos = nc.dram_tensor("cos", [128, S], BF16, kind="ExternalInput")
    sin = nc.dram_tensor("sin", [128, S], BF16, kind="ExternalInput")
    rot = nc.dram_tensor("rot", [128, 128], BF16, kind="ExternalInput")
    yout = nc.dram_tensor("y", [G, 128, S], BF16, kind="ExternalOutput")

    with tile.TileContext(nc) as tc:
        with (
            tc.tile_pool(name="const", bufs=1) as const,
            tc.tile_pool(name="sb", bufs=3) as sb,
            tc.tile_pool(name="ps", bufs=1, space="PSUM") as ps,
            tc.tile_pool(name="dram", bufs=1, space="DRAM") as dram,
        ):
            # ---- resident constants ----
            # wq chunks first on the scalar queue (needed immediately by the
            # si=0 projections), wo behind them (first needed ~60us in);
            # wk/wv/rope tables on the gpsimd queue; x streams on sync
            wq_sb = const.tile([128, KO, G * 128], BF16)
            wk_sb = const.tile([128, KO, 128], BF16)
            wv_sb = const.tile([128, KO, 128], BF16)
            wq_edges = [0, 1, 2, 3, 4] + list(range(6, KO + 1, 2))
            for ci in range(len(wq_edges) - 1):
                ksl = slice(wq_edges[ci], wq_edges[ci + 1])
                nc.scalar.dma_start(wq_sb[:, ksl, :], wq[:, ksl, :])
                if ci % 2 == 0 and ci // 2 < 8:
                    ksl2 = slice((ci // 2) * (KO // 8),
                                 (ci // 2 + 1) * (KO // 8))
                    nc.gpsimd.dma_start(wk_sb[:, ksl2, :], wk[:, ksl2, :])
                    nc.gpsimd.dma_start(wv_sb[:, ksl2, :], wv[:, ksl2, :])
            wo_sb = const.tile([128, MO, G * 128], BF16)
            bq_sb = const.tile([128, G], F32)
            bk_sb = const.tile([128, 1], F32)
            bv_sb = const.tile([128, 1], F32)
            nc.gpsimd.dma_start(bq_sb[:], bq[:, :])
            nc.gpsimd.dma_start(bk_sb[:], bk[:, :])
            nc.gpsimd.dma_start(bv_sb[:], bv[:, :])
            rot_sb = const.tile([128, 128], BF16)
            nc.gpsimd.dma_start(rot_sb[:], rot[:, :])
            cos_sb = const.tile([128, S], BF16)
            sin_sb = const.tile([128, S], BF16)
            nc.gpsimd.dma_start(cos_sb[:], cos[:, :])
            nc.gpsimd.dma_start(sin_sb[:], sin[:, :])
            ones_f = const.tile([128, 128], F32)
            nc.any.memset(ones_f[:], 1.0)
            ones_b = const.tile([128, 128], BF16)
            nc.vector.tensor_copy(ones_b[:], ones_f[:])
            ident = const.tile([128, 128], F32)
            make_identity(nc, ident)

            # ---- resident activations ----
            k_rot = const.tile([128, S], BF16)          # K, (d, l) layout
            v_t = const.tile([128, S // 128, 128], BF16)  # V^T (l%128, l//128, d)
            out_t = [[const.tile([128, ST], BF16, name=f"out_{g}_{si}")
                      for si in range(NST)] for g in range(G)]
            cc_in = dram.tile([MO, 128, S], BF16)
            cc_out = dram.tile([NCHUNK, 128, S], BF16)

            def sixtile(name):
                return ps.tile([128, ST], F32, tag="six", bufs=6, name=name)

            # ---------- o_proj filler units ----------
            # one unit = one (mo, si_col) y tile: 4 accumulating matmuls off
            # resident wo + the s-tile's attention outputs; pure PE filler
            evict_ctr = [0]

            def oproj_unit(mo, si_col, engines=("vector",),
                           dma_engines=("gpsimd", "scalar")):
                ps_y = sixtile("ps_y")
                for g in range(G):
                    nc.tensor.matmul(ps_y[:],
                                     wo_sb[:, mo, g * 128:(g + 1) * 128],
                                     out_t[g][si_col][:],
                                     start=(g == 0), stop=(g == G - 1))
                y_sb = sb.tile([128, ST], BF16, tag="y_sb", bufs=6)
                i = evict_ctr[0]
                evict_ctr[0] += 1
                eng = engines[i % len(engines)]
                if eng == "scalar":
                    nc.scalar.activation(y_sb[:], ps_y[:], ActF.Copy)
                else:
                    getattr(nc, eng).tensor_copy(y_sb[:], ps_y[:])
                dst = cc_in[mo][:, si_col * ST:(si_col + 1) * ST]
                getattr(nc, dma_engines[i % len(dma_engines)]).dma_start(
                    dst, y_sb[:])

            # ---------- QKV projection for one s-tile ----------
            def prefetch_x(si):
                sl = slice(si * ST, (si + 1) * ST)
                xts = []
                for ko in range(KO):
                    xt = sb.tile([128, ST], BF16, tag="x", bufs=KO)
                    nc.sync.dma_start(xt[:], x[ko][:, sl])
                    xts.append(xt)
                return xts

            def proj(si, xts):
                ps_q = [sixtile(f"ps_q{g}") for g in range(G)]
                ps_k = sixtile("ps_k")
                ps_v = sixtile("ps_v")
                for ko in range(KO):
                    xt = xts[ko]
                    st = (ko == 0)
                    sp = (ko == KO - 1)
                    for g in range(G):
                        nc.tensor.matmul(ps_q[g][:],
                                         wq_sb[:, ko, g * 128:(g + 1) * 128],
                                         xt[:], start=st, stop=sp)
                    nc.tensor.matmul(ps_k[:], wk_sb[:, ko, :], xt[:],
                                     start=st, stop=sp)
                    nc.tensor.matmul(ps_v[:], wv_sb[:, ko, :], xt[:],
                                     start=st, stop=sp)
                return ps_q, ps_k, ps_v

            # ---------- bias + RoPE (+ V transpose) for one s-tile ----------
            def ropes(si, ps_q, ps_k, ps_v, spare=None):
                sl = slice(si * ST, (si + 1) * ST)
                # evict all six accumulators on the scalar engine in ring
                # order so the "six" slots free for rope scratch in the same
                # order the ring reuses them
                q_raws = []
                for g in range(G):
                    q_raw = sb.tile([128, ST], BF16, tag="q_raw", bufs=4,
                                    name=f"q_raw{g}")
                    if g % 2 == 0:
                        nc.scalar.activation(q_raw[:], ps_q[g][:],
                                             ActF.Identity,
                                             bias=bq_sb[:, g:g + 1],
                                             scale=INV_SQRT_D)
                    else:
                        # bq is pre-scaled host-side: q = ps*inv + bq_s
                        nc.vector.tensor_scalar(q_raw[:], ps_q[g][:],
                                                INV_SQRT_D,
                                                bq_sb[:, g:g + 1],
                                                Alu.mult, Alu.add)
                    q_raws.append(q_raw)
                k_raw = sb.tile([128, ST], BF16, tag="k_raw", bufs=2)
                nc.vector.tensor_scalar(k_raw[:], ps_k[:], bk_sb[:, 0:1],
                                        None, Alu.add)
                v_sb = sb.tile([128, ST], F32, tag="v_sb", bufs=2)
                nc.scalar.activation(v_sb[:], ps_v[:], ActF.Identity,
                                     bias=bv_sb[:, 0:1])

                def drain_one(spare):
                    if spare:
                        oproj_unit(*spare.pop(0))

                def rope(raw_b, dst_ap):
                    ps_rot = sixtile("ps_rot")
                    nc.tensor.matmul(ps_rot[:], rot_sb[:], raw_b[:],
                                     start=True, stop=True)
                    t1 = sb.tile([128, ST], BF16, tag="rope_t1", bufs=2)
                    t2 = sb.tile([128, ST], BF16, tag="rope_t2", bufs=2)
                    nc.vector.tensor_tensor(t1[:], raw_b[:], cos_sb[:, sl],
                                            Alu.mult)
                    nc.vector.tensor_tensor(t2[:], ps_rot[:], sin_sb[:, sl],
                                            Alu.mult)
                    nc.vector.tensor_tensor(dst_ap, t1[:], t2[:], Alu.add)

                # PE order: q0 rope first (unblocks attention head 0), then K
                spare = list(spare or [])
                q_rots = []
                for g in range(G):
                    q_rot = sb.tile([128, ST], BF16, tag="q_rot", bufs=4,
                                    name=f"q_rot{g}")
                    rope(q_raws[g], q_rot[:])
                    drain_one(spare)
                    q_rots.append(q_rot)
                    if g == 0:
                        rope(k_raw, k_rot[:, sl])
                        drain_one(spare)
                for j in range(ST // 128):
                    ps_t = sixtile("ps_t")[:, 0:128]
                    nc.tensor.transpose(ps_t, v_sb[:, j * 128:(j + 1) * 128],
                                        ident[:])
                    nc.vector.tensor_copy(v_t[:, si * (ST // 128) + j, :],
                                          ps_t)
                while spare:
                    drain_one(spare)
                return q_rots

            # ---------- attention for one s-tile, with filler units ----------
            def attention(si, q_rots, fillers):
                nli = (si + 1) * (ST // 128)
                nblocks = G * nli
                b = 0
                emitted = 0

                # the normalize tail (den ones-matmul -> approx recip -> mult)
                # is deferred into the NEXT head's block stream so it never
                # blocks the in-order PE queue while the den chain drains
                def normalize(g, den_full, av_sb):
                    ps_den = sixtile("ps_den")
                    nc.tensor.matmul(ps_den[:], ones_b[:], den_full[:],
                                     start=True, stop=True)
                    recip = sb.tile([128, ST], F32, tag="recip", bufs=2)
                    nc.vector.reciprocal_approx_fast(out=recip[:],
                                                     in_=ps_den[:])
                    nc.vector.tensor_tensor(out_t[g][si][:], av_sb[:],
                                            recip[:], Alu.mult)

                pending = None
                for g in range(G):
                    q_rot = q_rots[g]
                    ps_av = ps.tile([128, ST], F32, tag="av", bufs=2)
                    # softmax denominator: two parallel bf16 accumulation
                    # chains — vector sums the full (below-diagonal) blocks,
                    # gpsimd the 4 diagonal-band blocks (first has off=0)
                    ndiag = min(nli, ST // 128)
                    nfull = nli - ndiag
                    den_a = (sb.tile([128, ST], BF16, tag="den_a", bufs=2,
                                     name="den_a")
                             if nfull > 0 else None)
                    den_b = sb.tile([128, ST], BF16, tag="den_b", bufs=2)
                    for li in range(nli):
                        j = li - si * (ST // 128)
                        off = 128 * j if j > 0 else 0
                        s2 = sixtile("s2")
                        nc.tensor.matmul(s2[:, off:],
                                         k_rot[:, li * 128:(li + 1) * 128],
                                         q_rot[:, off:], start=True, stop=True)
                        p = sb.tile([128, ST], BF16, tag="p", bufs=6)
                        nc.scalar.activation(p[:, off:], s2[:, off:], ActF.Exp)
                        if j >= 0:
                            # causal: triangular mask only touches the 128-col
                            # diagonal sub-block (keep s >= l)
                            nc.gpsimd.affine_select(
                                out=p[:, off:off + 128],
                                in_=p[:, off:off + 128],
                                compare_op=Alu.is_ge, fill=0.0,
                                base=0, channel_multiplier=-1,
                                pattern=[[1, 128]],
                            )
                        # interleave o_proj filler between the scores and
                        # AV matmuls so the exp round-trip is covered by
                        # queued PE work, including at head starts
                        b += 1
                        want = (b * len(fillers)) // nblocks
                        while emitted < want:
                            oproj_unit(*fillers[emitted])
                            emitted += 1
                        nc.tensor.matmul(ps_av[:, off:], v_t[:, li, :],
                                         p[:, off:],
                                         start=(li == 0), stop=(li == nli - 1))
                        if li < nfull:
                            if li == 0:
                                nc.vector.tensor_copy(den_a[:], p[:])
                            else:
                                nc.vector.tensor_tensor(den_a[:], den_a[:],
                                                        p[:], Alu.add)
                        else:
                            if li == nfull:
                                nc.gpsimd.tensor_copy(den_b[:], p[:])
                            else:
                                nc.gpsimd.tensor_tensor(den_b[:, off:],
                                                        den_b[:, off:],
                                                        p[:, off:], Alu.add)
                        # flush the previous head's deferred normalize
                        if pending is not None and li == min(4, nli - 1):
                            normalize(*pending)
                            pending = None
                    # merge den chains; evict av so its PSUM slot recycles
                    if nfull > 0:
                        den_full = sb.tile([128, ST], BF16, tag="den_f",
                                           bufs=2)
                        nc.vector.tensor_tensor(den_full[:], den_a[:],
                                                den_b[:], Alu.add)
                    else:
                        den_full = den_b
                    av_sb = sb.tile([128, ST], F32, tag="av_sb", bufs=4)
                    nc.vector.tensor_copy(av_sb[:], ps_av[:])
                    pending = (g, den_full, av_sb)
                if pending is not None:
                    normalize(*pending)
                    pending = None
                while emitted < len(fillers):
                    oproj_unit(*fillers[emitted])
                    emitted += 1

            # ---------- main pipeline ----------
            xts = prefetch_x(0)
            for c8 in range(8):
                msl = slice(c8 * (MO // 8), (c8 + 1) * (MO // 8))
                nc.sync.dma_start(wo_sb[:, msl, :], wo[:, msl, :])
            pq, pk, pv = proj(0, xts)
            q_rots = ropes(0, pq, pk, pv)
            for si in range(NST):
                if si + 1 < NST:
                    xts = prefetch_x(si + 1)
                fillers = ([(mo, si - 1) for mo in range(MO)]
                           if si >= 1 else [])
                nres = 4 if (fillers and si + 1 < NST) else 0
                spare = fillers[len(fillers) - nres:] if nres else []
                attention(si, q_rots, fillers[:len(fillers) - nres])
                if si + 1 < NST:
                    pq, pk, pv = proj(si + 1, xts)
                    q_rots = ropes(si + 1, pq, pk, pv, spare)

            # ---------- final o_proj column + chunked output ----------
            for chunk in range(NCHUNK):
                for mo in range(chunk * MO_PER, (chunk + 1) * MO_PER):
                    oproj_unit(mo, NST - 1,
                               engines=("vector", "scalar"),
                               dma_engines=("gpsimd", "scalar"))
                if with_collective:
                    # core c receives row-block mo = chunk*8 + c
                    nc.gpsimd.collective_compute(
                        "ReduceScatter",
                        Alu.add,
                        replica_groups=[list(range(NCORES))],
                        ins=[cc_in[chunk * MO_PER:(chunk + 1) * MO_PER].opt()],
                        outs=[cc_out[chunk:chunk + 1].opt()],
                    )
                    nc.sync.dma_start(yout[chunk:chunk + 1],
                                      cc_out[chunk:chunk + 1])
                else:
                    # profiling-only variant: local per-chunk copy instead of
                    # the collective (same yout DMA shape; output is an
                    # unreduced local shard)
                    srow = chunk * MO_PER + chunk
                    nc.sync.dma_start(yout[chunk:chunk + 1],
                                      cc_in[srow:srow + 1])

    nc.compile()
    return nc


def _rot_matrix():
    # q_rot = R @ q with rotate_half along D: R @ v = concat(-v[64:], v[:64])
    R = np.zeros((128, 128), np.float32)
    for i in range(64):
        R[i, 64 + i] = -1.0
        R[64 + i, i] = 1.0
    return R


def _prep_in_maps(inputs):
    x = np.ascontiguousarray(np.asarray(inputs["hidden_states"],
                                        np.float32)[0, :, 0, :])
    wq = np.asarray(inputs["wq"], np.float32)
    wk = np.asarray(inputs["wk"], np.float32)
    wv = np.asarray(inputs["wv"], np.float32)
    wo = np.asarray(inputs["wo"], np.float32)
    bq = np.asarray(inputs["bq"], np.float32) * INV_SQRT_D  # folded q scale
    bk = np.asarray(inputs["bk"], np.float32)
    bv = np.asarray(inputs["bv"], np.float32)
    cos_t = np.ascontiguousarray(
        np.asarray(inputs["cos_t"], np.float32)[0, 0]).astype(NBF)  # (128, S)
    sin_t = np.ascontiguousarray(
        np.asarray(inputs["sin_t"], np.float32)[0, 0]).astype(NBF)
    rotT = np.ascontiguousarray(_rot_matrix().T).astype(NBF)

    x_r = np.ascontiguousarray(x.reshape(KO, 128, S)).astype(NBF)
    in_maps = []
    for c in range(NCORES):
        qs = slice(c * G * 128, (c + 1) * G * 128)
        ks = slice(c * 128, (c + 1) * 128)
        wq_t = np.ascontiguousarray(
            wq[qs].T.reshape(KO, 128, G * 128).transpose(1, 0, 2)).astype(NBF)
        wk_t = np.ascontiguousarray(
            wk[ks].T.reshape(KO, 128, 128).transpose(1, 0, 2)).astype(NBF)
        wv_t = np.ascontiguousarray(
            wv[ks].T.reshape(KO, 128, 128).transpose(1, 0, 2)).astype(NBF)
        # wo column shard -> (d, mo, g*128+m): woT[g*128+d, mo*128+m]
        wo_t = np.ascontiguousarray(
            wo[:, qs].T.reshape(G, 128, MO, 128).transpose(1, 2, 0, 3)
            .reshape(128, MO, G * 128)).astype(NBF)
        in_maps.append({
            "x": x_r,
            "wq": wq_t,
            "wk": wk_t,
            "wv": wv_t,
            "wo": wo_t,
            "bq": np.ascontiguousarray(bq[qs].reshape(G, 128).T),
            "bk": np.ascontiguousarray(bk[ks][:, None]),
            "bv": np.ascontiguousarray(bv[ks][:, None]),
            "cos": cos_t,
            "sin": sin_t,
            "rot": rotT,
        })
    return in_maps


_NC = None


def _get_nc():
    global _NC
    if _NC is None:
        _NC = build_nc()
    return _NC


def assemble_output(results):
    """Chunked ReduceScatter: core c's chunk i is y row-block mo = 8*i + c."""
    y = np.empty((HID, S), np.float32)
    for c in range(NCORES):
        yc = results[c]["y"]
        for i in range(yc.shape[0]):
            mo = NCORES * i + c
            y[mo * 128:(mo + 1) * 128] = yc[i].astype(np.float32)
    return y[None, :, None, :]


def kernel(**inputs):
    nc = _get_nc()
    in_maps = _prep_in_maps(inputs)
    res = run_bass_kernel_spmd(nc, in_maps, core_ids=list(range(NCORES)))
    return assemble_output(res.results)


# revision 20
# speedup vs baseline: 1.0902x; 1.0902x over previous
"""Trainium2 Bass kernel for AttentionPatcher (GQA attention block, S=2048).

Sharding: 8-way tensor parallel over KV head groups. Core c owns KV head c
and query heads 4c..4c+3: it computes its Q/K/V projections, RoPE, causal
attention, and a full partial o_proj (wo column shard); a ReduceScatter(add)
over the 8 cores then leaves core c with rows [512c, 512c+512) of the final
output, which the host concatenates.

Schedule: all matmul operands are bf16 (PSUM accumulation stays fp32); wq
and wo stay resident in SBUF. o_proj is computed column-wise (a y s-column
only needs that s-tile's attention outputs), and its matmuls are
interleaved into the NEXT s-tile's attention stream as dependency-free
filler so the PE never stalls on the softmax exp round-trip. The softmax
denominator is accumulated across l-blocks on the vector/gpsimd engines
with a single ones-matmul per (head, s-tile).

PSUM layout: one 6-slot ring ("six") shared by the QKV projection
accumulators / score tiles / o_proj tiles / rope scratch, plus 2 dedicated
slots ("av") for the attention AV accumulators = exactly 8 banks.
"""
import os
import sys

import numpy as np
import ml_dtypes

if os.path.isdir("/opt/trn_rl_repo") and "/opt/trn_rl_repo" not in sys.path:
    sys.path.insert(0, "/opt/trn_rl_repo")

import concourse.bacc as bacc
import concourse.mybir as mybir
import concourse.tile as tile
from concourse.masks import make_identity
from concourse.bass_utils import run_bass_kernel_spmd

F32 = mybir.dt.float32
BF16 = mybir.dt.bfloat16
ActF = mybir.ActivationFunctionType
Alu = mybir.AluOpType
NBF = ml_dtypes.bfloat16

H, KV, D, S = 32, 8, 128, 2048
HID = H * D
NCORES = 8
G = H // KV          # query heads per core
ST = 512             # s-tile size
NST = S // ST        # 4 s-tiles
KO = HID // 128      # 32 contraction subtiles
MO = HID // 128      # 32 output row tiles
INV_SQRT_D = 1.0 / float(np.sqrt(D))
NCHUNK = 4           # ReduceScatter chunks of 8 row-blocks
MO_PER = MO // NCHUNK


def build_nc(with_collective=True):
    nc = bacc.Bacc("TRN2", target_bir_lowering=False, debug=False)

    x = nc.dram_tensor("x", [KO, 128, S], BF16, kind="ExternalInput")
    wq = nc.dram_tensor("wq", [128, KO, G * 128], BF16, kind="ExternalInput")
    wk = nc.dram_tensor("wk", [128, KO, 128], BF16, kind="ExternalInput")
    wv = nc.dram_tensor("wv", [128, KO, 128], BF16, kind="ExternalInput")
    wo = nc.dram_tensor("wo", [128, MO, G * 128], BF16, kind="ExternalInput")
    bq = nc.dram_tensor("bq", [128, G], F32, kind="ExternalInput")
    bk = nc.dram_tensor("bk", [128, 1], F32, kind="ExternalInput")
    bv = nc.dram_tensor("bv", [128, 1], F32, kind="ExternalInput")
    cos = nc.dram_tensor("cos", [128, S], BF16, kind="ExternalInput")
    sin = nc.dram_tensor("sin", [128, S], BF16, kind="ExternalInput")
    rot = nc.dram_tensor("rot", [128, 128], BF16, kind="ExternalInput")
    yout = nc.dram_tensor("y", [G, 128, S], BF16, kind="ExternalOutput")

    with tile.TileContext(nc) as tc:
        with (
            tc.tile_pool(name="const", bufs=1) as const,
            tc.tile_pool(name="sb", bufs=3) as sb,
            tc.tile_pool(name="ps", bufs=1, space="PSUM") as ps,
            tc.tile_pool(name="dram", bufs=1, space="DRAM") as dram,
        ):
            # ---- resident constants ----
            # wq chunks first on the scalar queue (needed immediately by the
            # si=0 projections), wo behind them (first needed ~60us in);
            # wk/wv/rope tables on the gpsimd queue; x streams on sync
            wq_sb = const.tile([128, KO, G * 128], BF16)
            wk_sb = const.tile([128, KO, 128], BF16)
            wv_sb = const.tile([128, KO, 128], BF16)
            wq_edges = [0, 1, 2, 3, 4] + list(range(6, KO + 1, 2))
            for ci in range(len(wq_edges) - 1):
                ksl = slice(wq_edges[ci], wq_edges[ci + 1])
                nc.scalar.dma_start(wq_sb[:, ksl, :], wq[:, ksl, :])
                if ci % 2 == 0 and ci // 2 < 8:
                    ksl2 = slice((ci // 2) * (KO // 8),
                                 (ci // 2 + 1) * (KO // 8))
                    nc.gpsimd.dma_start(wk_sb[:, ksl2, :], wk[:, ksl2, :])
                    nc.gpsimd.dma_start(wv_sb[:, ksl2, :], wv[:, ksl2, :])
            wo_sb = const.tile([128, MO, G * 128], BF16)
            bq_sb = const.tile([128, G], F32)
            bk_sb = const.tile([128, 1], F32)
            bv_sb = const.tile([128, 1], F32)
            nc.gpsimd.dma_start(bq_sb[:], bq[:, :])
            nc.gpsimd.dma_start(bk_sb[:], bk[:, :])
            nc.gpsimd.dma_start(bv_sb[:], bv[:, :])
            rot_sb = const.tile([128, 128], BF16)
            nc.gpsimd.dma_start(rot_sb[:], rot[:, :])
            cos_sb = const.tile([128, S], BF16)
            sin_sb = const.tile([128, S], BF16)
            nc.gpsimd.dma_start(cos_sb[:], cos[:, :])
            nc.gpsimd.dma_start(sin_sb[:], sin[:, :])
            ones_f = const.tile([128, 128], F32)
            nc.any.memset(ones_f[:], 1.0)
            ones_b = const.tile([128, 128], BF16)
            nc.vector.tensor_copy(ones_b[:], ones_f[:])
            ident = const.tile([128, 128], F32)
            make_identity(nc, ident)

            # ---- resident activations ----
            k_rot = const.tile([128, S], BF16)          # K, (d, l) layout
            v_t = const.tile([128, S // 128, 128], BF16)  # V^T (l%128, l//128, d)
            out_t = [[const.tile([128, ST], BF16, name=f"out_{g}_{si}")
                      for si in range(NST)] for g in range(G)]
            cc_in = dram.tile([MO, 128, S], BF16)
            cc_out = dram.tile([NCHUNK, 128, S], BF16)

            def sixtile(name):
                return ps.tile([128, ST], F32, tag="six", bufs=6, name=name)

            # ---------- o_proj filler units ----------
            # one unit = one (mo, si_col) y tile: 4 accumulating matmuls off
            # resident wo + the s-tile's attention outputs; pure PE filler
            evict_ctr = [0]

            def oproj_unit(mo, si_col, engines=("vector",),
                           dma_engines=("gpsimd", "scalar")):
                ps_y = sixtile("ps_y")
                for g in range(G):
                    nc.tensor.matmul(ps_y[:],
                                     wo_sb[:, mo, g * 128:(g + 1) * 128],
                                     out_t[g][si_col][:],
                                     start=(g == 0), stop=(g == G - 1))
                y_sb = sb.tile([128, ST], BF16, tag="y_sb", bufs=6)
                i = evict_ctr[0]
                evict_ctr[0] += 1
                eng = engines[i % len(engines)]
                if eng == "scalar":
                    nc.scalar.activation(y_sb[:], ps_y[:], ActF.Copy)
                else:
                    getattr(nc, eng).tensor_copy(y_sb[:], ps_y[:])
                dst = cc_in[mo][:, si_col * ST:(si_col + 1) * ST]
                getattr(nc, dma_engines[i % len(dma_engines)]).dma_start(
                    dst, y_sb[:])

            # ---------- QKV projection for one s-tile ----------
            def prefetch_x(si):
                sl = slice(si * ST, (si + 1) * ST)
                xts = []
                for ko in range(KO):
                    xt = sb.tile([128, ST], BF16, tag="x", bufs=KO)
                    nc.sync.dma_start(xt[:], x[ko][:, sl])
                    xts.append(xt)
                return xts

            def proj(si, xts):
                ps_q = [sixtile(f"ps_q{g}") for g in range(G)]
                ps_k = sixtile("ps_k")
                ps_v = sixtile("ps_v")
                for ko in range(KO):
                    xt = xts[ko]
                    st = (ko == 0)
                    sp = (ko == KO - 1)
                    for g in range(G):
                        nc.tensor.matmul(ps_q[g][:],
                                         wq_sb[:, ko, g * 128:(g + 1) * 128],
                                         xt[:], start=st, stop=sp)
                    nc.tensor.matmul(ps_k[:], wk_sb[:, ko, :], xt[:],
                                     start=st, stop=sp)
                    nc.tensor.matmul(ps_v[:], wv_sb[:, ko, :], xt[:],
                                     start=st, stop=sp)
                return ps_q, ps_k, ps_v

            # ---------- bias + RoPE (+ V transpose) for one s-tile ----------
            def ropes(si, ps_q, ps_k, ps_v, spare=None):
                sl = slice(si * ST, (si + 1) * ST)
                # evict all six accumulators on the scalar engine in ring
                # order so the "six" slots free for rope scratch in the same
                # order the ring reuses them
                q_raws = []
                for g in range(G):
                    q_raw = sb.tile([128, ST], BF16, tag="q_raw", bufs=4,
                                    name=f"q_raw{g}")
                    if g % 2 == 0:
                        nc.scalar.activation(q_raw[:], ps_q[g][:],
                                             ActF.Identity,
                                             bias=bq_sb[:, g:g + 1],
                                             scale=INV_SQRT_D)
                    else:
                        # bq is pre-scaled host-side: q = ps*inv + bq_s
                        nc.vector.tensor_scalar(q_raw[:], ps_q[g][:],
                                                INV_SQRT_D,
                                                bq_sb[:, g:g + 1],
                                                Alu.mult, Alu.add)
                    q_raws.append(q_raw)
                k_raw = sb.tile([128, ST], BF16, tag="k_raw", bufs=2)
                nc.vector.tensor_scalar(k_raw[:], ps_k[:], bk_sb[:, 0:1],
                                        None, Alu.add)
                v_sb = sb.tile([128, ST], F32, tag="v_sb", bufs=2)
                nc.scalar.activation(v_sb[:], ps_v[:], ActF.Identity,
                                     bias=bv_sb[:, 0:1])

                def drain_one(spare):
                    if spare:
                        oproj_unit(*spare.pop(0))

                def rope(raw_b, dst_ap):
                    ps_rot = sixtile("ps_rot")
                    nc.tensor.matmul(ps_rot[:], rot_sb[:], raw_b[:],
                                     start=True, stop=True)
                    t1 = sb.tile([128, ST], BF16, tag="rope_t1", bufs=2)
                    t2 = sb.tile([128, ST], BF16, tag="rope_t2", bufs=2)
                    nc.vector.tensor_tensor(t1[:], raw_b[:], cos_sb[:, sl],
                                            Alu.mult)
                    nc.vector.tensor_tensor(t2[:], ps_rot[:], sin_sb[:, sl],
                                            Alu.mult)
                    nc.vector.tensor_tensor(dst_ap, t1[:], t2[:], Alu.add)

                # PE order: q0 rope first (unblocks attention head 0), then K
                spare = list(spare or [])
                q_rots = []
                for g in range(G):
                    q_rot = sb.tile([128, ST], BF16, tag="q_rot", bufs=4,
                                    name=f"q_rot{g}")
                    rope(q_raws[g], q_rot[:])
                    drain_one(spare)
                    q_rots.append(q_rot)
                    if g == 0:
                        rope(k_raw, k_rot[:, sl])
                        drain_one(spare)
                for j in range(ST // 128):
                    ps_t = sixtile("ps_t")[:, 0:128]
                    nc.tensor.transpose(ps_t, v_sb[:, j * 128:(j + 1) * 128],
                                        ident[:])
                    nc.vector.tensor_copy(v_t[:, si * (ST // 128) + j, :],
                                          ps_t)
                while spare:
                    drain_one(spare)
                return q_rots

            # ---------- attention for one s-tile, with filler units ----------
            def attention(si, q_rots, fillers):
                nli = (si + 1) * (ST // 128)
                nblocks = G * nli
                b = 0
                emitted = 0

                # the normalize tail (den ones-matmul -> approx recip -> mult)
                # is deferred into the NEXT head's block stream so it never
                # blocks the in-order PE queue while the den chain drains
                def normalize(g, den_full, av_sb):
                    ps_den = sixtile("ps_den")
                    nc.tensor.matmul(ps_den[:], ones_b[:], den_full[:],
                                     start=True, stop=True)
                    recip = sb.tile([128, ST], F32, tag="recip", bufs=2)
                    nc.vector.reciprocal_approx_fast(out=recip[:],
                                                     in_=ps_den[:])
                    nc.vector.tensor_tensor(out_t[g][si][:], av_sb[:],
                                            recip[:], Alu.mult)

                pending = None
                for g in range(G):
                    q_rot = q_rots[g]
                    ps_av = ps.tile([128, ST], F32, tag="av", bufs=2)
                    # softmax denominator: two parallel bf16 accumulation
                    # chains — vector sums the full (below-diagonal) blocks,
                    # gpsimd the 4 diagonal-band blocks (first has off=0)
                    ndiag = min(nli, ST // 128)
                    nfull = nli - ndiag
                    den_a = (sb.tile([128, ST], BF16, tag="den_a", bufs=2,
                                     name="den_a")
                             if nfull > 0 else None)
                    den_b = sb.tile([128, ST], BF16, tag="den_b", bufs=2)
                    for li in range(nli):
                        j = li - si * (ST // 128)
                        off = 128 * j if j > 0 else 0
                        s2 = sixtile("s2")
                        nc.tensor.matmul(s2[:, off:],
                                         k_rot[:, li * 128:(li + 1) * 128],
                                         q_rot[:, off:], start=True, stop=True)
                        p = sb.tile([128, ST], BF16, tag="p", bufs=6)
                        nc.scalar.activation(p[:, off:], s2[:, off:], ActF.Exp)
                        if j >= 0:
                            # causal: triangular mask only touches the 128-col
                            # diagonal sub-block (keep s >= l)
                            nc.gpsimd.affine_select(
                                out=p[:, off:off + 128],
                                in_=p[:, off:off + 128],
                                compare_op=Alu.is_ge, fill=0.0,
                                base=0, channel_multiplier=-1,
                                pattern=[[1, 128]],
                            )
                        # interleave o_proj filler between the scores and
                        # AV matmuls so the exp round-trip is covered by
                        # queued PE work, including at head starts
                        b += 1
                        want = (b * len(fillers)) // nblocks
                        while emitted < want:
                            oproj_unit(*fillers[emitted])
                            emitted += 1
                        nc.tensor.matmul(ps_av[:, off:], v_t[:, li, :],
                                         p[:, off:],
                                         start=(li == 0), stop=(li == nli - 1))
                        if li < nfull:
                            if li == 0:
                                nc.vector.tensor_copy(den_a[:], p[:])
                            else:
                                nc.vector.tensor_tensor(den_a[:], den_a[:],
                                                        p[:], Alu.add)
                        else:
                            if li == nfull:
                                nc.vector.tensor_copy(den_b[:], p[:])
                            else:
                                nc.vector.tensor_tensor(den_b[:, off:],
                                                        den_b[:, off:],
                                                        p[:, off:], Alu.add)
                        # flush the previous head's deferred normalize
                        if pending is not None and li == min(4, nli - 1):
                            normalize(*pending)
                            pending = None
                    # merge den chains; evict av so its PSUM slot recycles
                    if nfull > 0:
                        den_full = sb.tile([128, ST], BF16, tag="den_f",
                                           bufs=2)
                        nc.vector.tensor_tensor(den_full[:], den_a[:],
                                                den_b[:], Alu.add)
                    else:
                        den_full = den_b
                    av_sb = sb.tile([128, ST], F32, tag="av_sb", bufs=4)
                    nc.vector.tensor_copy(av_sb[:], ps_av[:])
                    pending = (g, den_full, av_sb)
                if pending is not None:
                    normalize(*pending)
                    pending = None
                while emitted < len(fillers):
                    oproj_unit(*fillers[emitted])
                    emitted += 1

            # ---------- main pipeline ----------
            xts = prefetch_x(0)
            for c8 in range(8):
                msl = slice(c8 * (MO // 8), (c8 + 1) * (MO // 8))
                nc.sync.dma_start(wo_sb[:, msl, :], wo[:, msl, :])
            pq, pk, pv = proj(0, xts)
            q_rots = ropes(0, pq, pk, pv)
            for si in range(NST):
                if si + 1 < NST:
                    xts = prefetch_x(si + 1)
                fillers = ([(mo, si - 1) for mo in range(MO)]
                           if si >= 1 else [])
                nres = 4 if (fillers and si + 1 < NST) else 0
                spare = fillers[len(fillers) - nres:] if nres else []
                attention(si, q_rots, fillers[:len(fillers) - nres])
                if si + 1 < NST:
                    pq, pk, pv = proj(si + 1, xts)
                    q_rots = ropes(si + 1, pq, pk, pv, spare)

            # ---------- final o_proj column + chunked output ----------
            for chunk in range(NCHUNK):
                for mo in range(chunk * MO_PER, (chunk + 1) * MO_PER):
                    oproj_unit(mo, NST - 1,
                               engines=("vector", "scalar"),
                               dma_engines=("gpsimd", "scalar"))
                if with_collective:
                    # core c receives row-block mo = chunk*8 + c
                    nc.gpsimd.collective_compute(
                        "ReduceScatter",
                        Alu.add,
                        replica_groups=[list(range(NCORES))],
                        ins=[cc_in[chunk * MO_PER:(chunk + 1) * MO_PER].opt()],
                        outs=[cc_out[chunk:chunk + 1].opt()],
                    )
                    nc.sync.dma_start(yout[chunk:chunk + 1],
                                      cc_out[chunk:chunk + 1])
                else:
                    # profiling-only variant: local per-chunk copy instead of
                    # the collective (same yout DMA shape; output is an
                    # unreduced local shard)
                    srow = chunk * MO_PER + chunk
                    nc.sync.dma_start(yout[chunk:chunk + 1],
                                      cc_in[srow:srow + 1])

    nc.compile()
    return nc


def _rot_matrix():
    # q_rot = R @ q with rotate_half along D: R @ v = concat(-v[64:], v[:64])
    R = np.zeros((128, 128), np.float32)
    for i in range(64):
        R[i, 64 + i] = -1.0
        R[64 + i, i] = 1.0
    return R


def _prep_in_maps(inputs):
    x = np.ascontiguousarray(np.asarray(inputs["hidden_states"],
                                        np.float32)[0, :, 0, :])
    wq = np.asarray(inputs["wq"], np.float32)
    wk = np.asarray(inputs["wk"], np.float32)
    wv = np.asarray(inputs["wv"], np.float32)
    wo = np.asarray(inputs["wo"], np.float32)
    bq = np.asarray(inputs["bq"], np.float32) * INV_SQRT_D  # folded q scale
    bk = np.asarray(inputs["bk"], np.float32)
    bv = np.asarray(inputs["bv"], np.float32)
    cos_t = np.ascontiguousarray(
        np.asarray(inputs["cos_t"], np.float32)[0, 0]).astype(NBF)  # (128, S)
    sin_t = np.ascontiguousarray(
        np.asarray(inputs["sin_t"], np.float32)[0, 0]).astype(NBF)
    rotT = np.ascontiguousarray(_rot_matrix().T).astype(NBF)

    x_r = np.ascontiguousarray(x.reshape(KO, 128, S)).astype(NBF)
    in_maps = []
    for c in range(NCORES):
        qs = slice(c * G * 128, (c + 1) * G * 128)
        ks = slice(c * 128, (c + 1) * 128)
        wq_t = np.ascontiguousarray(
            wq[qs].T.reshape(KO, 128, G * 128).transpose(1, 0, 2)).astype(NBF)
        wk_t = np.ascontiguousarray(
            wk[ks].T.reshape(KO, 128, 128).transpose(1, 0, 2)).astype(NBF)
        wv_t = np.ascontiguousarray(
            wv[ks].T.reshape(KO, 128, 128).transpose(1, 0, 2)).astype(NBF)
        # wo column shard -> (d, mo, g*128+m): woT[g*128+d, mo*128+m]
        wo_t = np.ascontiguousarray(
            wo[:, qs].T.reshape(G, 128, MO, 128).transpose(1, 2, 0, 3)
            .reshape(128, MO, G * 128)).astype(NBF)
        in_maps.append({
            "x": x_r,
            "wq": wq_t,
            "wk": wk_t,
            "wv": wv_t,
            "wo": wo_t,
            "bq": np.ascontiguousarray(bq[qs].reshape(G, 128).T),
            "bk": np.ascontiguousarray(bk[ks][:, None]),
            "bv": np.ascontiguousarray(bv[ks][:, None]),
            "cos": cos_t,
            "sin": sin_t,
            "rot": rotT,
        })
    return in_maps


_NC = None


def _get_nc():
    global _NC
    if _NC is None:
        _NC = build_nc()
    return _NC


def assemble_output(results):
    """Chunked ReduceScatter: core c's chunk i is y row-block mo = 8*i + c."""
    y = np.empty((HID, S), np.float32)
    for c in range(NCORES):
        yc = results[c]["y"]
        for i in range(yc.shape[0]):
            mo = NCORES * i + c
            y[mo * 128:(mo + 1) * 128] = yc[i].astype(np.float32)
    return y[None, :, None, :]


def kernel(**inputs):
    nc = _get_nc()
    in_maps = _prep_in_maps(inputs)
    res = run_bass_kernel_spmd(nc, in_maps, core_ids=list(range(NCORES)))
    return assemble_output(res.results)
